# revision 20
# baseline (speedup 1.0000x reference)
"""Trainium2 Bass kernel for nn_BaseTransformer (ensemble member-attention block).

Sharding: data-parallel over batch B=8 across 8 NeuronCores (1 batch each).
Weights/constants replicated. No collectives.

Reference math (per batch b, x = in_tensor[b] as [K=16, C=64, S=4096]):
  value = einsum('ics,oc->ios', x, Wv)
  key   = selu(einsum(x, Wk)); query = selu(einsum(x, Wq))
  gram[c,i,j] = sum_s key[i,c,s] query[j,c,s] / 64        (then * lambda^2 fold)
  A = softmax(gram, axis=i) + I
  transformed[j] = sum_i (A[c,i,j] - 1/16) value_i        (exact mean fold)
  out = selu(x + einsum(transformed, w_out) + b_out)

Layout/dtype scheme (v2):
  - x_bf16 resident as 8 pair tiles [128, S] (members t, t+8); feeds the
    transposed k/q conv (x chunks as PE stationary operand -> k,q come out
    [s, heads]) and the value conv. x_fp32 is re-streamed from HBM in phase 2
    only for the exact residual add.
  - stride-8 head groups (head c = 8u+g) so gram operands are single-stride
    APs and the value gather/scatter DMAs use contiguous partition runs
    (sigma = bit-swap permutation folded into Wv columns / Wout rows).
  - selu(t) = min(alpha*e^t - alpha, relu(t)) composed exactly from
    ACT Exp (bias ln a), ACT Relu, DVE scalar_tensor_tensor (sub/min).
  - mix matmul is block-diagonal over 8 heads x 16 members with the
    B matrices assembled via permutation matmuls (P^T (softmax masked)^T P').
"""

import sys

if "/opt/trn_rl_repo" not in sys.path:
    sys.path.insert(0, "/opt/trn_rl_repo")

import numpy as np

import concourse.bass as bass
import concourse.bacc as bacc
import concourse.mybir as mybir
import concourse.tile as tile

F32 = mybir.dt.float32
BF16 = mybir.dt.bfloat16

K, C, HEADS, S = 16, 64, 64, 4096
NG = 8           # head groups of 8 (stride-8: group g = heads {8u+g})
SC1 = 128        # phase-1 spatial chunk (gram contraction tile)
NCH1 = S // SC1  # 32
SC2 = 512        # phase-2 spatial chunk
NCH2 = S // SC2  # 8

ALPHA = 1.6732632423543772
LAMBDA = 1.0507009873554805
LN_ALPHA = float(np.log(ALPHA))
LN_LAMBDA_ALPHA = float(np.log(LAMBDA * ALPHA))
GRAM_SCALE = float(LAMBDA * LAMBDA / 64.0)


def _pi(u, i):
    return 64 * (i // 8) + 8 * u + (i % 8)


def host_constants(w_value, w_key, w_query, w_out, b_out):
    """Build all replicated device inputs on the host."""
    consts = {}
    # sigma: head c = 8u+g  <->  storage position 8g+u (group-contiguous).
    sigma = np.zeros(64, np.int64)
    for u in range(8):
        for g in range(8):
            sigma[8 * g + u] = 8 * u + g
    wvT = np.ascontiguousarray(w_value.T[:, sigma])
    consts["wvT"] = np.concatenate([wvT, wvT], axis=0).astype(np.float32)
    wkqT = np.ascontiguousarray(np.concatenate([w_key.T, w_query.T], axis=1))
    consts["wkqT"] = np.concatenate([wkqT, wkqT], axis=0).astype(np.float32)
    woutT = np.ascontiguousarray(w_out.T[sigma, :])
    consts["woutT"] = np.concatenate([woutT, woutT], axis=0).astype(np.float32)

    # Gram psum layout: partition = 8j+u (q side), free = 8i+u' (k side).
    # MASK zeroes cross-head entries (u != u').
    mask = np.zeros((128, 128), np.float32)
    for p in range(128):
        for f in range(128):
            if p % 8 == f % 8:
                mask[p, f] = 1.0
    consts["maskg"] = mask

    # P (mm2 lhsT): rows r=(i,u)=8i+u -> out partition pi(u, i); same matrix
    # serves as P' (mm1 rhs) for the j side.
    P = np.zeros((128, 128), np.float32)
    for u in range(8):
        for i in range(16):
            P[8 * i + u, _pi(u, i)] = 1.0
    consts["permP"] = P
    consts["permPp"] = P.copy()

    # DPAT in permuted coords: D[pi(u,i), pi(u,j)] = delta(i,j) - 1/16.
    D = np.zeros((128, 128), np.float32)
    for u in range(8):
        for i in range(16):
            for j in range(16):
                D[_pi(u, i), _pi(u, j)] = (1.0 if i == j else 0.0) - 1.0 / 16.0
    consts["dpat"] = D

    consts["b_out_col"] = np.concatenate([b_out, b_out]).astype(
        np.float32).reshape(128, 1)
    return consts


def build_nc():
    """Build the single-core Bass program (same NEFF on all 8 cores)."""
    nc = bacc.Bacc("TRN2", target_bir_lowering=False, debug=False)

    x_d = nc.dram_tensor("x", [K, C, S], F32, kind="ExternalInput")
    wvT_d = nc.dram_tensor("wvT", [128, 64], F32, kind="ExternalInput")
    wkqT_d = nc.dram_tensor("wkqT", [128, 128], F32, kind="ExternalInput")
    woutT_d = nc.dram_tensor("woutT", [128, 64], F32, kind="ExternalInput")
    mask_d = nc.dram_tensor("maskg", [128, 128], F32, kind="ExternalInput")
    permP_d = nc.dram_tensor("permP", [128, 128], F32, kind="ExternalInput")
    permPp_d = nc.dram_tensor("permPp", [128, 128], F32, kind="ExternalInput")
    dpat_d = nc.dram_tensor("dpat", [128, 128], F32, kind="ExternalInput")
    bo_d = nc.dram_tensor("b_out_col", [128, 1], F32, kind="ExternalInput")
    out_d = nc.dram_tensor("out", [K, C, S], F32, kind="ExternalOutput")

    with tile.TileContext(nc) as tc:
        with (
            tc.tile_pool(name="persist", bufs=1) as persist,
            tc.tile_pool(name="xpool", bufs=1) as xpool,
        ):
            # ---- weights / constants to SBUF (+ bf16 casts) ----
            wv_f = persist.tile([128, 64], F32, tag="wvf")
            nc.sync.dma_start(out=wv_f, in_=wvT_d[:, :])
            wv_sb = persist.tile([128, 64], BF16, tag="wv")
            nc.gpsimd.tensor_copy(wv_sb, wv_f)
            wkq_f = persist.tile([128, 128], F32, tag="wkqf")
            nc.sync.dma_start(out=wkq_f, in_=wkqT_d[:, :])
            wkq_sb = persist.tile([128, 128], BF16, tag="wkq")
            nc.gpsimd.tensor_copy(wkq_sb, wkq_f)
            wo_f = persist.tile([128, 64], F32, tag="wof")
            nc.sync.dma_start(out=wo_f, in_=woutT_d[:, :])
            wo_sb = persist.tile([128, 64], BF16, tag="wo")
            nc.gpsimd.tensor_copy(wo_sb, wo_f)
            mask_sb = persist.tile([128, 128], F32, tag="mask")
            nc.sync.dma_start(out=mask_sb, in_=mask_d[:, :])
            permP_sb = persist.tile([128, 128], F32, tag="permP")
            nc.sync.dma_start(out=permP_sb, in_=permP_d[:, :])
            permPp_sb = persist.tile([128, 128], F32, tag="permPp")
            nc.sync.dma_start(out=permPp_sb, in_=permPp_d[:, :])
            dpat_sb = persist.tile([128, 128], F32, tag="dpat")
            nc.sync.dma_start(out=dpat_sb, in_=dpat_d[:, :])
            bo_sb = persist.tile([128, 1], F32, tag="bo")
            nc.sync.dma_start(out=bo_sb, in_=bo_d[:, :])
            lna_sb = persist.tile([128, 1], F32, tag="lna")
            nc.vector.memset(lna_sb, LN_ALPHA)
            lnla_sb = persist.tile([128, 1], F32, tag="lnla")
            nc.vector.memset(lnla_sb, LN_LAMBDA_ALPHA)
            zero_sb = persist.tile([128, 1], F32, tag="zero")
            nc.vector.memset(zero_sb, 0.0)

            # ---- x: stream fp32, cast to resident bf16 pair tiles ----
            x_sb = []
            with tc.tile_pool(name="xload", bufs=3) as xload:
                for t in range(8):
                    xf = xload.tile([128, S], F32, tag="xf")
                    nc.sync.dma_start(out=xf[0:64, :], in_=x_d[t, :, :])
                    nc.sync.dma_start(out=xf[64:128, :], in_=x_d[t + 8, :, :])
                    xb = xpool.tile([128, S], BF16, tag=f"x{t}")
                    nc.gpsimd.tensor_copy(xb, xf)
                    x_sb.append(xb)

            # BigB result tiles (persist into phase 2), bf16 for the mix matmul
            bigB = []
            for g in range(NG):
                bigB_t = persist.tile([128, 128], BF16, tag=f"bigB{g}")
                bigB.append(bigB_t)

            # =========================== PHASE 1 ===========================
            with (
                tc.tile_pool(name="p1sb", bufs=3) as p1sb,
                tc.tile_pool(name="p1sc", bufs=3) as p1sc,
                tc.tile_pool(name="kqps", bufs=2, space="PSUM") as kqps,
                tc.tile_pool(name="gramps", bufs=1, space="PSUM") as gramps,
            ):
                gram_ps = []
                for gb in range(2):
                    gram_t = gramps.tile([128, 512], F32, tag=f"gram{gb}")
                    gram_ps.append(gram_t)

                for sc in range(NCH1):
                    sl = slice(SC1 * sc, SC1 * (sc + 1))
                    # kqT free layout: half*1024 + m*64 + c   (bf16)
                    kqT = p1sb.tile([128, K * 128], BF16, tag="kqT")
                    for blk in range(2):  # member blocks [0..8), [8..16)
                        ps = kqps.tile([128, 8 * 128], F32, tag="kqps")
                        for mb in range(8):
                            m = blk * 8 + mb
                            xt = x_sb[m % 8]
                            rhalf = slice(0, 64) if m < 8 else slice(64, 128)
                            nc.tensor.matmul(
                                ps[:, 128 * mb: 128 * (mb + 1)],
                                xt[rhalf, sl], wkq_sb[rhalf, :],
                                start=True, stop=True,
                            )
                        # selu: e2 = exp(kq + ln a); r = relu(kq);
                        # out = (e2 - a) min r   (all bf16 outputs)
                        e2 = p1sc.tile([128, 8 * 128], BF16, tag="e2")
                        nc.scalar.activation(
                            out=e2, in_=ps,
                            func=mybir.ActivationFunctionType.Exp,
                            bias=lna_sb[:, 0:1])
                        r = p1sc.tile([128, 8 * 128], BF16, tag="r")
                        nc.scalar.activation(
                            out=r, in_=ps,
                            func=mybir.ActivationFunctionType.Relu,
                            bias=zero_sb[:, 0:1])
                        ev = e2.rearrange("p (mb half c) -> p half mb c",
                                          mb=8, half=2, c=64)
                        rv = r.rearrange("p (mb half c) -> p half mb c",
                                         mb=8, half=2, c=64)
                        for half in range(2):
                            nc.vector.scalar_tensor_tensor(
                                out=kqT[:, 1024 * half + 512 * blk:
                                        1024 * half + 512 * (blk + 1)],
                                in0=ev[:, half], scalar=ALPHA, in1=rv[:, half],
                                op0=mybir.AluOpType.subtract,
                                op1=mybir.AluOpType.min)
                    # gram: lhsT = q side (M = 8j+u), rhs = k side (N = 8i+u'),
                    # single-stride [[8,128]] APs at offset g
                    vq = kqT.rearrange("p (f e) -> p e f", f=256, e=8)
                    for g in range(NG):
                        q_ap = vq[:, g, 128:256]
                        k_ap = vq[:, g, 0:128]
                        nc.tensor.matmul(
                            gram_ps[g // 4][:, 128 * (g % 4): 128 * (g % 4 + 1)],
                            q_ap, k_ap,
                            start=(sc == 0 and g % 4 == 0),
                            stop=(sc == NCH1 - 1 and g % 4 == 3))

                # ---- softmax (no max-sub; range pre-verified) + BigB ----
                for g in range(NG):
                    gp = gram_ps[g // 4][:, 128 * (g % 4): 128 * (g % 4 + 1)]
                    E = p1sc.tile([128, 128], F32, tag="E")
                    nc.scalar.activation(
                        out=E, in_=gp,
                        func=mybir.ActivationFunctionType.Exp,
                        bias=zero_sb[:, 0:1], scale=GRAM_SCALE)
                    Ssum = p1sc.tile([128, 8], F32, tag="Ssum")
                    nc.vector.tensor_reduce(
                        out=Ssum,
                        in_=E.rearrange("p (i u) -> p u i", i=16, u=8),
                        axis=mybir.AxisListType.X, op=mybir.AluOpType.add)
                    R = p1sc.tile([128, 8], F32, tag="R")
                    nc.vector.reciprocal(out=R, in_=Ssum)
                    Eu = E.rearrange("p (i u) -> p u i", i=16, u=8)
                    for u in range(8):
                        nc.vector.tensor_scalar(
                            out=Eu[:, u, :], in0=Eu[:, u, :],
                            scalar1=R[:, u: u + 1], scalar2=None,
                            op0=mybir.AluOpType.mult)
                    nc.vector.tensor_tensor(
                        out=E, in0=E, in1=mask_sb, op=mybir.AluOpType.mult)
                    c_ps = kqps.tile([128, 128], F32, tag="kqps")
                    nc.tensor.matmul(c_ps, E, permPp_sb, start=True, stop=True)
                    c_sb = p1sc.tile([128, 128], F32, tag="permcsb")
                    nc.scalar.copy(c_sb, c_ps)
                    b_ps = kqps.tile([128, 128], F32, tag="kqps")
                    nc.tensor.matmul(b_ps, permP_sb, c_sb, start=True, stop=True)
                    nc.vector.scalar_tensor_tensor(
                        out=bigB[g], in0=b_ps, scalar=1.0, in1=dpat_sb,
                        op0=mybir.AluOpType.mult, op1=mybir.AluOpType.add)

            # =========================== PHASE 2 ===========================
            with (
                tc.tile_pool(name="vflat", bufs=2) as vflatp,
                tc.tile_pool(name="tflat", bufs=2) as tflatp,
                tc.tile_pool(name="xsp", bufs=3) as xsp,
                tc.tile_pool(name="p2sc", bufs=4) as p2sc,
                tc.tile_pool(name="p2out", bufs=3) as p2outp,
                tc.tile_pool(name="vps", bufs=3, space="PSUM") as vps,
                tc.tile_pool(name="mps", bufs=3, space="PSUM") as mps,
                tc.tile_pool(name="ops", bufs=2, space="PSUM") as ops,
            ):
                gathers_hist = [[], []]    # per-slot gather DMAs (bufs=2)
                conv2_hist = [None, None]  # last conv2 matmul per tflat slot
                for pc in range(NCH2):
                    sl = slice(SC2 * pc, SC2 * (pc + 1))
                    slot = pc % 2
                    # --- value conv into [128,1024] psum (2 pairs), copy/cast ---
                    # vflat2 [128, 8*SC2] bf16: row = 64*(i//8) + sigma-pos(c),
                    # free = (i%8)*SC2 + s
                    vflat2 = vflatp.tile([128, 8 * SC2], BF16, tag="vflat2")
                    vcopies = []
                    for t in range(8):
                        ps = vps.tile([128, SC2], F32, tag="vps")
                        nc.tensor.matmul(
                            ps[0:64, :], wv_sb[0:64, :], x_sb[t][0:64, sl],
                            start=True, stop=True)
                        nc.tensor.matmul(
                            ps[64:128, :], wv_sb[64:128, :], x_sb[t][64:128, sl],
                            start=True, stop=True)
                        vci = nc.vector.tensor_copy(
                            vflat2[:, SC2 * t: SC2 * (t + 1)], ps)
                        vcopies.append(vci)
                        # WAR: this slot's previous gathers read the old tile
                        if t == 0:
                            for gi_prev in gathers_hist[slot]:
                                tile.add_dep_helper(
                                    vci.ins, gi_prev.ins,
                                    reason="vflat2 WAR vs prev gathers")
                    gathers_hist[slot] = []
                    # --- mix: gather -> blockdiag matmul -> copy -> scatter ---
                    tflat2 = tflatp.tile([128, 8 * SC2], BF16, tag="tflat2")
                    vv = vflat2.rearrange("p (it s) -> p it s", it=8, s=SC2)
                    tv = tflat2.rearrange("p (jt s) -> p jt s", jt=8, s=SC2)
                    scatters = []
                    for g in range(NG):
                        pm = mps.tile([128, SC2], F32, tag="mps")
                        vg = p2sc.tile([128, SC2], BF16, tag="vg")
                        for i2 in range(2):
                            base = 64 * i2 + 8 * g
                            gi = nc.sync.dma_start(
                                out=vg[64 * i2: 64 * (i2 + 1), :],
                                in_=vv[base: base + 8, :, :])
                            tile.add_dep_helper(
                                gi.ins, vcopies[-1].ins,
                                reason="gather after value copies")
                            gathers_hist[slot].append(gi)
                        nc.tensor.matmul(pm, bigB[g], vg, start=True, stop=True)
                        mg = p2sc.tile([128, SC2], BF16, tag="mg")
                        nc.scalar.copy(mg, pm)
                        for j2 in range(2):
                            base = 64 * j2 + 8 * g
                            si = nc.sync.dma_start(
                                out=tv[base: base + 8, :, :],
                                in_=mg[64 * j2: 64 * (j2 + 1), :])
                            scatters.append(si)
                            if conv2_hist[slot] is not None:
                                tile.add_dep_helper(
                                    si.ins, conv2_hist[slot].ins,
                                    reason="tflat2 WAR vs prev conv2")
                    # --- conv2, exact fp32 residual add (+b_out), final selu ---
                    for jt in range(8):
                        xs = xsp.tile([128, SC2], F32, tag="xs")
                        nc.sync.dma_start(out=xs[0:64, :], in_=x_d[jt, :, sl])
                        nc.sync.dma_start(out=xs[64:128, :],
                                          in_=x_d[jt + 8, :, sl])
                        po = ops.tile([128, SC2], F32, tag="ops")
                        mi = nc.tensor.matmul(
                            po[0:64, :], wo_sb[0:64, :],
                            tflat2[0:64, SC2 * jt: SC2 * (jt + 1)],
                            start=True, stop=True)
                        if jt == 0:
                            for si in scatters:
                                tile.add_dep_helper(
                                    mi.ins, si.ins,
                                    reason="conv2 after scatters")
                        mi2 = nc.tensor.matmul(
                            po[64:128, :], wo_sb[64:128, :],
                            tflat2[64:128, SC2 * jt: SC2 * (jt + 1)],
                            start=True, stop=True)
                        conv2_hist[slot] = mi2
                        # y = po + b_out + x   (exact fp32 residual)
                        ty = p2sc.tile([128, SC2], F32, tag="ty")
                        nc.vector.scalar_tensor_tensor(
                            out=ty, in0=po, scalar=bo_sb[:, 0:1], in1=xs,
                            op0=mybir.AluOpType.add, op1=mybir.AluOpType.add)
                        # selu(y) = min(l*a*e^y - l*a, l*relu(y))
                        e2f = p2sc.tile([128, SC2], F32, tag="fe2")
                        nc.scalar.activation(
                            out=e2f, in_=ty,
                            func=mybir.ActivationFunctionType.Exp,
                            bias=lnla_sb[:, 0:1])
                        r2f = p2sc.tile([128, SC2], F32, tag="fr2")
                        nc.gpsimd.tensor_scalar(
                            out=r2f, in0=ty, scalar1=0.0, scalar2=LAMBDA,
                            op0=mybir.AluOpType.max, op1=mybir.AluOpType.mult)
                        o_sb = p2outp.tile([128, SC2], F32, tag="osb")
                        nc.vector.scalar_tensor_tensor(
                            out=o_sb, in0=e2f, scalar=float(LAMBDA * ALPHA),
                            in1=r2f,
                            op0=mybir.AluOpType.subtract,
                            op1=mybir.AluOpType.min)
                        nc.sync.dma_start(out=out_d[jt, :, sl],
                                          in_=o_sb[0:64, :])
                        nc.sync.dma_start(out=out_d[jt + 8, :, sl],
                                          in_=o_sb[64:128, :])
    nc.compile()
    return nc


_NC_CACHE = None


def _get_nc():
    global _NC_CACHE
    if _NC_CACHE is None:
        _NC_CACHE = build_nc()
    return _NC_CACHE


def kernel(in_tensor, w_value, w_key, w_query, w_out, b_out, **_ignored):
    in_tensor = np.asarray(in_tensor, dtype=np.float32)
    w_value = np.asarray(w_value, dtype=np.float32)
    w_key = np.asarray(w_key, dtype=np.float32)
    w_query = np.asarray(w_query, dtype=np.float32)
    w_out = np.asarray(w_out, dtype=np.float32)
    b_out = np.asarray(b_out, dtype=np.float32)

    B = in_tensor.shape[0]
    assert B == 8
    consts = host_constants(w_value, w_key, w_query, w_out, b_out)

    nc = _get_nc()
    in_maps = []
    for b in range(B):
        m = {"x": np.ascontiguousarray(in_tensor[b].reshape(K, C, S))}
        m.update(consts)
        in_maps.append(m)

    from concourse.bass_utils import run_bass_kernel_spmd

    res = run_bass_kernel_spmd(nc, in_maps, core_ids=list(range(8)))
    outs = [res.results[b]["out"].reshape(K, C, 64, 64) for b in range(B)]
    return np.stack(outs, axis=0).astype(np.float32)


if __name__ == "__main__":
    build_nc()
    print("built ok")


# revision 23
# speedup vs baseline: 1.1902x; 1.1902x over previous
"""Trainium2 Bass kernel for nn_BaseTransformer (ensemble member-attention block).

Sharding: data-parallel over batch B=8 across 8 NeuronCores (1 batch each).
Weights/constants replicated. No collectives.

Reference math (per batch b, x = in_tensor[b] as [K=16, C=64, S=4096]):
  value = einsum('ics,oc->ios', x, Wv)
  key   = selu(einsum(x, Wk)); query = selu(einsum(x, Wq))
  gram[c,i,j] = sum_s key[i,c,s] query[j,c,s] / 64        (then * lambda^2 fold)
  A = softmax(gram, axis=i) + I
  transformed[j] = sum_i (A[c,i,j] - 1/16) value_i        (exact mean fold)
  out = selu(x + einsum(transformed, w_out) + b_out)

Layout/dtype scheme (v2):
  - x_bf16 resident as 8 pair tiles [128, S] (members t, t+8); feeds the
    transposed k/q conv (x chunks as PE stationary operand -> k,q come out
    [s, heads]) and the value conv. x_fp32 is re-streamed from HBM in phase 2
    only for the exact residual add.
  - stride-8 head groups (head c = 8u+g) so gram operands are single-stride
    APs and the value gather/scatter DMAs use contiguous partition runs
    (sigma = bit-swap permutation folded into Wv columns / Wout rows).
  - selu(t) = min(alpha*e^t - alpha, relu(t)) composed exactly from
    ACT Exp (bias ln a), ACT Relu, DVE scalar_tensor_tensor (sub/min).
  - mix matmul is block-diagonal over 8 heads x 16 members with the
    B matrices assembled via permutation matmuls (P^T (softmax masked)^T P').
"""

import sys

if "/opt/trn_rl_repo" not in sys.path:
    sys.path.insert(0, "/opt/trn_rl_repo")

import numpy as np

import concourse.bass as bass
import concourse.bacc as bacc
import concourse.mybir as mybir
import concourse.tile as tile

F32 = mybir.dt.float32
BF16 = mybir.dt.bfloat16

K, C, HEADS, S = 16, 64, 64, 4096
NG = 8           # head groups of 8 (stride-8: group g = heads {8u+g})
SC1 = 128        # phase-1 spatial chunk (gram contraction tile)
NCH1 = S // SC1  # 32
SC2 = 512        # phase-2 spatial chunk
NCH2 = S // SC2  # 8

ALPHA = 1.6732632423543772
LAMBDA = 1.0507009873554805
LN_ALPHA = float(np.log(ALPHA))
LN_LAMBDA_ALPHA = float(np.log(LAMBDA * ALPHA))
GRAM_SCALE = float(LAMBDA * LAMBDA / 64.0)


def _pi(u, i):
    return 64 * (i // 8) + 8 * u + (i % 8)


def host_constants(w_value, w_key, w_query, w_out, b_out):
    """Build all replicated device inputs on the host."""
    consts = {}
    # sigma: head c = 8u+g  <->  storage position 8g+u (group-contiguous).
    sigma = np.zeros(64, np.int64)
    for u in range(8):
        for g in range(8):
            sigma[8 * g + u] = 8 * u + g
    wvT = np.ascontiguousarray(w_value.T[:, sigma])
    consts["wvT"] = np.concatenate([wvT, wvT], axis=0).astype(np.float32)
    wkqT = np.ascontiguousarray(np.concatenate([w_key.T, w_query.T], axis=1))
    consts["wkqT"] = np.concatenate([wkqT, wkqT], axis=0).astype(np.float32)
    woutT = np.ascontiguousarray(w_out.T[sigma, :])
    consts["woutT"] = np.concatenate([woutT, woutT], axis=0).astype(np.float32)

    # Gram psum layout: partition = 8j+u (q side), free = 8i+u' (k side).
    # MASK zeroes cross-head entries (u != u').
    mask = np.zeros((128, 128), np.float32)
    for p in range(128):
        for f in range(128):
            if p % 8 == f % 8:
                mask[p, f] = 1.0
    consts["maskg"] = mask

    # P (mm2 lhsT): rows r=(i,u)=8i+u -> out partition pi(u, i); same matrix
    # serves as P' (mm1 rhs) for the j side.
    P = np.zeros((128, 128), np.float32)
    for u in range(8):
        for i in range(16):
            P[8 * i + u, _pi(u, i)] = 1.0
    consts["permP"] = P
    consts["permPp"] = P.copy()

    # DPAT in permuted coords: D[pi(u,i), pi(u,j)] = delta(i,j) - 1/16.
    D = np.zeros((128, 128), np.float32)
    for u in range(8):
        for i in range(16):
            for j in range(16):
                D[_pi(u, i), _pi(u, j)] = (1.0 if i == j else 0.0) - 1.0 / 16.0
    consts["dpat"] = D

    consts["b_out_col"] = np.concatenate([b_out, b_out]).astype(
        np.float32).reshape(128, 1)
    return consts


def build_nc():
    """Build the single-core Bass program (same NEFF on all 8 cores)."""
    nc = bacc.Bacc("TRN2", target_bir_lowering=False, debug=False)

    x_d = nc.dram_tensor("x", [K, C, S], F32, kind="ExternalInput")
    wvT_d = nc.dram_tensor("wvT", [128, 64], F32, kind="ExternalInput")
    wkqT_d = nc.dram_tensor("wkqT", [128, 128], F32, kind="ExternalInput")
    woutT_d = nc.dram_tensor("woutT", [128, 64], F32, kind="ExternalInput")
    mask_d = nc.dram_tensor("maskg", [128, 128], F32, kind="ExternalInput")
    permP_d = nc.dram_tensor("permP", [128, 128], F32, kind="ExternalInput")
    permPp_d = nc.dram_tensor("permPp", [128, 128], F32, kind="ExternalInput")
    dpat_d = nc.dram_tensor("dpat", [128, 128], F32, kind="ExternalInput")
    bo_d = nc.dram_tensor("b_out_col", [128, 1], F32, kind="ExternalInput")
    out_d = nc.dram_tensor("out", [K, C, S], F32, kind="ExternalOutput")

    with tile.TileContext(nc) as tc:
        with (
            tc.tile_pool(name="persist", bufs=1) as persist,
            tc.tile_pool(name="xpool", bufs=1) as xpool,
        ):
            # ---- weights / constants to SBUF (+ bf16 casts) ----
            wv_f = persist.tile([128, 64], F32, tag="wvf")
            nc.sync.dma_start(out=wv_f, in_=wvT_d[:, :])
            wv_sb = persist.tile([128, 64], BF16, tag="wv")
            nc.gpsimd.tensor_copy(wv_sb, wv_f)
            wkq_f = persist.tile([128, 128], F32, tag="wkqf")
            nc.sync.dma_start(out=wkq_f, in_=wkqT_d[:, :])
            wkq_sb = persist.tile([128, 128], BF16, tag="wkq")
            nc.gpsimd.tensor_copy(wkq_sb, wkq_f)
            wo_f = persist.tile([128, 64], F32, tag="wof")
            nc.sync.dma_start(out=wo_f, in_=woutT_d[:, :])
            wo_sb = persist.tile([128, 64], BF16, tag="wo")
            nc.gpsimd.tensor_copy(wo_sb, wo_f)
            mask_sb = persist.tile([128, 128], F32, tag="mask")
            nc.sync.dma_start(out=mask_sb, in_=mask_d[:, :])
            permP_sb = persist.tile([128, 128], F32, tag="permP")
            nc.sync.dma_start(out=permP_sb, in_=permP_d[:, :])
            permPp_sb = persist.tile([128, 128], F32, tag="permPp")
            nc.sync.dma_start(out=permPp_sb, in_=permPp_d[:, :])
            dpat_sb = persist.tile([128, 128], F32, tag="dpat")
            nc.sync.dma_start(out=dpat_sb, in_=dpat_d[:, :])
            bo_sb = persist.tile([128, 1], F32, tag="bo")
            nc.sync.dma_start(out=bo_sb, in_=bo_d[:, :])
            lna_sb = persist.tile([128, 1], F32, tag="lna")
            nc.vector.memset(lna_sb, LN_ALPHA)
            lnla_sb = persist.tile([128, 1], F32, tag="lnla")
            nc.vector.memset(lnla_sb, LN_LAMBDA_ALPHA)
            zero_sb = persist.tile([128, 1], F32, tag="zero")
            nc.vector.memset(zero_sb, 0.0)

            # ---- x: stream fp32, cast to resident bf16 pair tiles ----
            x_sb = []
            with tc.tile_pool(name="xload", bufs=3) as xload:
                for t in range(8):
                    xf = xload.tile([128, S], F32, tag="xf")
                    nc.sync.dma_start(out=xf[0:64, :], in_=x_d[t, :, :])
                    nc.sync.dma_start(out=xf[64:128, :], in_=x_d[t + 8, :, :])
                    xb = xpool.tile([128, S], BF16, tag=f"x{t}")
                    if t % 3 == 1:
                        nc.scalar.copy(xb, xf)
                    elif t % 3 == 2:
                        nc.gpsimd.tensor_copy(xb, xf)
                    else:
                        nc.vector.tensor_copy(xb, xf)
                    x_sb.append(xb)

            # BigB result tiles (persist into phase 2), bf16 for the mix matmul
            bigB = []
            for g in range(NG):
                bigB_t = persist.tile([128, 128], BF16, tag=f"bigB{g}")
                bigB.append(bigB_t)

            # =========================== PHASE 1 ===========================
            with (
                tc.tile_pool(name="p1sb", bufs=3) as p1sb,
                tc.tile_pool(name="p1sc", bufs=3) as p1sc,
                tc.tile_pool(name="kqps", bufs=2, space="PSUM") as kqps,
                tc.tile_pool(name="gramps", bufs=1, space="PSUM") as gramps,
            ):
                gram_ps = []
                for gb in range(2):
                    gram_t = gramps.tile([128, 512], F32, tag=f"gram{gb}")
                    gram_ps.append(gram_t)

                for sc in range(NCH1):
                    sl = slice(SC1 * sc, SC1 * (sc + 1))
                    # kqT free layout: half*1024 + m*64 + c   (bf16)
                    kqT = p1sb.tile([128, K * 128], BF16, tag="kqT")
                    for blk in range(2):  # member blocks [0..8), [8..16)
                        ps = kqps.tile([128, 8 * 128], F32, tag="kqps")
                        for mb in range(8):
                            m = blk * 8 + mb
                            xt = x_sb[m % 8]
                            rhalf = slice(0, 64) if m < 8 else slice(64, 128)
                            nc.tensor.matmul(
                                ps[:, 128 * mb: 128 * (mb + 1)],
                                xt[rhalf, sl], wkq_sb[rhalf, :],
                                start=True, stop=True,
                            )
                        # selu: e2 = exp(kq + ln a); r = relu(kq);
                        # out = (e2 - a) min r   (all bf16 outputs).
                        # e2/r stored half-split (h, mb, c) so the stt reads
                        # contiguous halves (DVE 2x bf16 mode).
                        e2 = p1sc.tile([128, 8 * 128], BF16, tag="e2")
                        e2v = e2.rearrange("p (h mb c) -> p mb h c",
                                           mb=8, h=2, c=64)
                        nc.scalar.activation(
                            out=e2v, in_=ps,
                            func=mybir.ActivationFunctionType.Exp,
                            bias=lna_sb[:, 0:1])
                        r = p1sc.tile([128, 8 * 128], BF16, tag="r")
                        rv = r.rearrange("p (h mb c) -> p mb h c",
                                         mb=8, h=2, c=64)
                        if (sc + blk) % 2 == 0:
                            nc.scalar.activation(
                                out=rv, in_=ps,
                                func=mybir.ActivationFunctionType.Relu,
                                bias=zero_sb[:, 0:1])
                        else:
                            nc.vector.tensor_scalar(
                                out=rv, in0=ps, scalar1=0.0, scalar2=None,
                                op0=mybir.AluOpType.max)
                        for half in range(2):
                            nc.vector.scalar_tensor_tensor(
                                out=kqT[:, 1024 * half + 512 * blk:
                                        1024 * half + 512 * (blk + 1)],
                                in0=e2[:, 512 * half: 512 * (half + 1)],
                                scalar=ALPHA,
                                in1=r[:, 512 * half: 512 * (half + 1)],
                                op0=mybir.AluOpType.subtract,
                                op1=mybir.AluOpType.min)
                    # gram: lhsT = q side (M = 8j+u), rhs = k side (N = 8i+u'),
                    # single-stride [[8,128]] APs at offset g
                    vq = kqT.rearrange("p (f e) -> p e f", f=256, e=8)
                    for g in range(NG):
                        q_ap = vq[:, g, 128:256]
                        k_ap = vq[:, g, 0:128]
                        nc.tensor.matmul(
                            gram_ps[g // 4][:, 128 * (g % 4): 128 * (g % 4 + 1)],
                            q_ap, k_ap,
                            start=(sc == 0 and g % 4 == 0),
                            stop=(sc == NCH1 - 1 and g % 4 == 3))

                # ---- softmax (no max-sub; range pre-verified) + BigB ----
                for g in range(NG):
                    gp = gram_ps[g // 4][:, 128 * (g % 4): 128 * (g % 4 + 1)]
                    E = p1sc.tile([128, 128], F32, tag="E")
                    nc.scalar.activation(
                        out=E, in_=gp,
                        func=mybir.ActivationFunctionType.Exp,
                        bias=zero_sb[:, 0:1], scale=GRAM_SCALE)
                    Ssum = p1sc.tile([128, 8], F32, tag="Ssum")
                    nc.vector.tensor_reduce(
                        out=Ssum,
                        in_=E.rearrange("p (i u) -> p u i", i=16, u=8),
                        axis=mybir.AxisListType.X, op=mybir.AluOpType.add)
                    R = p1sc.tile([128, 8], F32, tag="R")
                    nc.vector.reciprocal(out=R, in_=Ssum)
                    Eu = E.rearrange("p (i u) -> p u i", i=16, u=8)
                    for u in range(8):
                        nc.vector.tensor_scalar(
                            out=Eu[:, u, :], in0=Eu[:, u, :],
                            scalar1=R[:, u: u + 1], scalar2=None,
                            op0=mybir.AluOpType.mult)
                    nc.vector.tensor_tensor(
                        out=E, in0=E, in1=mask_sb, op=mybir.AluOpType.mult)
                    c_ps = kqps.tile([128, 128], F32, tag="kqps")
                    nc.tensor.matmul(c_ps, E, permPp_sb, start=True, stop=True)
                    c_sb = p1sc.tile([128, 128], F32, tag="permcsb")
                    nc.scalar.copy(c_sb, c_ps)
                    b_ps = kqps.tile([128, 128], F32, tag="kqps")
                    nc.tensor.matmul(b_ps, permP_sb, c_sb, start=True, stop=True)
                    nc.vector.scalar_tensor_tensor(
                        out=bigB[g], in0=b_ps, scalar=1.0, in1=dpat_sb,
                        op0=mybir.AluOpType.mult, op1=mybir.AluOpType.add)

            # =========================== PHASE 2 ===========================
            # Software-pipelined 3 stages deep: value(pc) | mix(pc-1) |
            # conv2+final(pc-2), so the in-order PE queue always has
            # independent matmuls while DMA/copy stages of older chunks drain.
            with (
                tc.tile_pool(name="vflat", bufs=2) as vflatp,
                tc.tile_pool(name="tflat", bufs=2) as tflatp,
                tc.tile_pool(name="xsp", bufs=4) as xsp,
                tc.tile_pool(name="p2sc", bufs=4) as p2sc,
                tc.tile_pool(name="p2out", bufs=3) as p2outp,
                tc.tile_pool(name="vps", bufs=3, space="PSUM") as vps,
                tc.tile_pool(name="mps", bufs=3, space="PSUM") as mps,
                tc.tile_pool(name="ops", bufs=2, space="PSUM") as ops,
            ):
                xv_d = x_d.rearrange("(m2 mt) c s -> mt m2 c s", m2=2, mt=8)
                ov_d = out_d.rearrange("(m2 mt) c s -> mt m2 c s", m2=2, mt=8)
                gathers_hist = [[], []]
                conv2_hist = [None, None]
                vstate = {}
                tstate = {}

                def stage_value(pc):
                    sl = slice(SC2 * pc, SC2 * (pc + 1))
                    slot = pc % 2
                    vflat2 = vflatp.tile([128, 8 * SC2], BF16, tag="vflat2")
                    vcopies = []
                    for t in range(8):
                        ps = vps.tile([128, SC2], F32, tag="vps")
                        nc.tensor.matmul(
                            ps[0:64, :], wv_sb[0:64, :], x_sb[t][0:64, sl],
                            start=True, stop=True)
                        nc.tensor.matmul(
                            ps[64:128, :], wv_sb[64:128, :], x_sb[t][64:128, sl],
                            start=True, stop=True)
                        vci = nc.vector.tensor_copy(
                            vflat2[:, SC2 * t: SC2 * (t + 1)], ps)
                        vcopies.append(vci)
                        if t == 0:
                            for gi_prev in gathers_hist[slot]:
                                tile.add_dep_helper(
                                    vci.ins, gi_prev.ins,
                                    reason="vflat2 WAR vs prev gathers")
                    gathers_hist[slot] = []
                    vstate[pc] = (vflat2, vcopies)

                def stage_mix(pc):
                    slot = pc % 2
                    vflat2, vcopies = vstate.pop(pc)
                    tflat2 = tflatp.tile([128, 8 * SC2], BF16, tag="tflat2")
                    vv = vflat2.rearrange("p (it s) -> p it s", it=8, s=SC2)
                    tv = tflat2.rearrange("p (jt s) -> p jt s", jt=8, s=SC2)
                    scatters = []
                    for g in range(NG):
                        pm = mps.tile([128, SC2], F32, tag="mps")
                        vg = p2sc.tile([128, SC2], BF16, tag="vg")
                        for i2 in range(2):
                            base = 64 * i2 + 8 * g
                            gi = nc.sync.dma_start(
                                out=vg[64 * i2: 64 * (i2 + 1), :],
                                in_=vv[base: base + 8, :, :])
                            tile.add_dep_helper(
                                gi.ins, vcopies[-1].ins,
                                reason="gather after value copies")
                            gathers_hist[slot].append(gi)
                        nc.tensor.matmul(pm, bigB[g], vg, start=True, stop=True)
                        mg = p2sc.tile([128, SC2], BF16, tag="mg")
                        nc.scalar.copy(mg, pm)
                        for j2 in range(2):
                            base = 64 * j2 + 8 * g
                            si = nc.sync.dma_start(
                                out=tv[base: base + 8, :, :],
                                in_=mg[64 * j2: 64 * (j2 + 1), :])
                            scatters.append(si)
                            if conv2_hist[slot] is not None:
                                tile.add_dep_helper(
                                    si.ins, conv2_hist[slot].ins,
                                    reason="tflat2 WAR vs prev conv2")
                    tstate[pc] = (tflat2, scatters)

                def stage_out(pc):
                    sl = slice(SC2 * pc, SC2 * (pc + 1))
                    slot = pc % 2
                    tflat2, scatters = tstate.pop(pc)
                    for jt in range(8):
                        xs = xsp.tile([128, SC2], F32, tag="xs")
                        nc.sync.dma_start(out=xs, in_=xv_d[jt, :, :, sl])
                        po = ops.tile([128, SC2], F32, tag="ops")
                        mi = nc.tensor.matmul(
                            po[0:64, :], wo_sb[0:64, :],
                            tflat2[0:64, SC2 * jt: SC2 * (jt + 1)],
                            start=True, stop=True)
                        if jt == 0:
                            for si in scatters:
                                tile.add_dep_helper(
                                    mi.ins, si.ins,
                                    reason="conv2 after scatters")
                        mi2 = nc.tensor.matmul(
                            po[64:128, :], wo_sb[64:128, :],
                            tflat2[64:128, SC2 * jt: SC2 * (jt + 1)],
                            start=True, stop=True)
                        conv2_hist[slot] = mi2
                        # y = po + b_out + x   (exact fp32 residual)
                        ty = p2sc.tile([128, SC2], F32, tag="ty")
                        nc.vector.scalar_tensor_tensor(
                            out=ty, in0=po, scalar=bo_sb[:, 0:1], in1=xs,
                            op0=mybir.AluOpType.add, op1=mybir.AluOpType.add)
                        # selu(y) = min(l*a*e^y - l*a, l*relu(y))
                        e2f = p2sc.tile([128, SC2], F32, tag="fe2")
                        nc.scalar.activation(
                            out=e2f, in_=ty,
                            func=mybir.ActivationFunctionType.Exp,
                            bias=lnla_sb[:, 0:1])
                        r2f = p2sc.tile([128, SC2], F32, tag="fr2")
                        nc.vector.tensor_scalar(
                            out=r2f, in0=ty, scalar1=0.0, scalar2=LAMBDA,
                            op0=mybir.AluOpType.max, op1=mybir.AluOpType.mult)
                        o_sb = p2outp.tile([128, SC2], F32, tag="osb")
                        nc.vector.scalar_tensor_tensor(
                            out=o_sb, in0=e2f, scalar=float(LAMBDA * ALPHA),
                            in1=r2f,
                            op0=mybir.AluOpType.subtract,
                            op1=mybir.AluOpType.min)
                        nc.sync.dma_start(out=ov_d[jt, :, :, sl], in_=o_sb)

                for pc in range(NCH2 + 2):
                    if pc < NCH2:
                        stage_value(pc)
                    if 1 <= pc <= NCH2:
                        stage_mix(pc - 1)
                    if pc >= 2:
                        stage_out(pc - 2)
    nc.compile()
    return nc


_NC_CACHE = None


def _get_nc():
    global _NC_CACHE
    if _NC_CACHE is None:
        _NC_CACHE = build_nc()
    return _NC_CACHE


def kernel(in_tensor, w_value, w_key, w_query, w_out, b_out, **_ignored):
    in_tensor = np.asarray(in_tensor, dtype=np.float32)
    w_value = np.asarray(w_value, dtype=np.float32)
    w_key = np.asarray(w_key, dtype=np.float32)
    w_query = np.asarray(w_query, dtype=np.float32)
    w_out = np.asarray(w_out, dtype=np.float32)
    b_out = np.asarray(b_out, dtype=np.float32)

    B = in_tensor.shape[0]
    assert B == 8
    consts = host_constants(w_value, w_key, w_query, w_out, b_out)

    nc = _get_nc()
    in_maps = []
    for b in range(B):
        m = {"x": np.ascontiguousarray(in_tensor[b].reshape(K, C, S))}
        m.update(consts)
        in_maps.append(m)

    from concourse.bass_utils import run_bass_kernel_spmd

    res = run_bass_kernel_spmd(nc, in_maps, core_ids=list(range(8)))
    outs = [res.results[b]["out"].reshape(K, C, 64, 64) for b in range(B)]
    return np.stack(outs, axis=0).astype(np.float32)


if __name__ == "__main__":
    build_nc()
    print("built ok")
